# revision 5
# baseline (speedup 1.0000x reference)
"""Trainium2 Bass kernel for ragged-sequence attention.

reference computation (per batch b):
    energy[s] = sum_d key[s,b,:] . query[b,:]          (S=2048, D=512)
    w = softmax(energy) * mask;  w /= max(sum(w), eps)
    context = sum_s w[s] * value[s,b,:]
returns (context (B,D), w (B,S)).

Sharding: pure data parallel over batch, 4 batches per core on 8 cores.
"""
import contextlib
import os

import numpy as np

S, B, D = 2048, 32, 512
NCORES = 8
BL = B // NCORES          # 4 local batches per core
P = 128                   # partitions
NT = S // P               # 16 s-tiles of 128
NG = 4                    # DMA groups: 4 s-tiles (1 MiB) per transfer
GJ = NT // NG             # s-tiles per group
EPS = 1e-12

_CACHE = {}
LAST_RESULT = None


def _build():
    from concourse import bacc, mybir
    import concourse.tile as tile

    F32 = mybir.dt.float32
    nc = bacc.Bacc("TRN2", target_bir_lowering=False, debug=False,
                   num_devices=NCORES)

    K_in = nc.dram_tensor("K_in", [S, BL, D], F32, kind="ExternalInput").ap()
    V_in = nc.dram_tensor("V_in", [S, BL, D], F32, kind="ExternalInput").ap()
    q_in = nc.dram_tensor("q_in", [1, BL * D], F32, kind="ExternalInput").ap()
    th_in = nc.dram_tensor("th_in", [NT, BL], F32, kind="ExternalInput").ap()
    io_in = nc.dram_tensor("io_in", [NT, P], F32, kind="ExternalInput").ap()
    id_in = nc.dram_tensor("id_in", [P, P], F32, kind="ExternalInput").ap()
    ctx_out = nc.dram_tensor("ctx_out", [BL, D], F32, kind="ExternalOutput").ap()
    w_out = nc.dram_tensor("w_out", [BL, NT, P], F32, kind="ExternalOutput").ap()

    # (g j p) b d -> g b p j d : tile (b,g) = [128p, 4j, 512d]
    Kv = K_in.rearrange("(g j p) b d -> g b p j d", g=NG, j=GJ, p=P)
    Vv = V_in.rearrange("(g j p) b d -> g b p j d", g=NG, j=GJ, p=P)

    with tile.TileContext(nc) as tc:
        with contextlib.ExitStack() as ctx:
            kp = ctx.enter_context(tc.tile_pool(name="kp", bufs=6))
            vp = ctx.enter_context(tc.tile_pool(name="vp", bufs=8))
            cb = ctx.enter_context(tc.tile_pool(name="cb", bufs=1))
            tp = ctx.enter_context(tc.tile_pool(name="tp", bufs=2))
            wp = ctx.enter_context(tc.tile_pool(name="wp", bufs=2))
            pp = ctx.enter_context(tc.tile_pool(name="pp", bufs=1, space="PSUM"))

            # --- constants (small queue: gpsimd SWDGE) ---
            ident = cb.tile([P, P], F32)
            nc.gpsimd.dma_start(ident[:], id_in[:])
            iota = cb.tile([NT, P], F32)
            nc.gpsimd.dma_start(iota[:], io_in[:])
            th = cb.tile([NT, BL], F32)
            nc.gpsimd.dma_start(th[:], th_in[:])
            qsb = cb.tile([1, BL * D], F32)
            nc.gpsimd.dma_start(qsb[:], q_in[:])
            ones = cb.tile([NT, P], F32)
            nc.gpsimd.memset(ones[:], 1.0)

            # --- q broadcast to [128, 512] per batch (PE) ---
            qb = []
            for b in range(BL):
                qb_ps = pp.tile([P, D], F32, tag="qb_ps")
                nc.tensor.matmul(qb_ps[:], ones[0:1, :], qsb[0:1, b * D:(b + 1) * D],
                                 start=True, stop=True)
                qb_b = cb.tile([P, D], F32, tag=f"qb{b}")
                nc.scalar.copy(qb_b[:], qb_ps[:])
                qb.append(qb_b)

            # --- big-stream DMAs, hand-interleaved on the sync (SP) ring ---
            # order: K0 K0 K0 K0 | K1 V0 K1 V0 K1 V0 K1 V0 | K2 V1 .. | K3 V2 .. | V3 x4
            ktiles = [[None] * NG for _ in range(BL)]
            vtiles = [[None] * NG for _ in range(BL)]

            def load_k(b, g):
                t = kp.tile([P, GJ, D], F32, tag="kt")
                nc.sync.dma_start(t[:], Kv[g, b])
                ktiles[b][g] = t

            def load_v(b, g):
                t = vp.tile([P, GJ, D], F32, tag="vt")
                nc.sync.dma_start(t[:], Vv[g, b])
                vtiles[b][g] = t

            for g in range(NG):
                load_k(0, g)
            for b in range(1, BL):
                for g in range(NG):
                    load_k(b, g)
                    load_v(b - 1, g)
            for g in range(NG):
                load_v(BL - 1, g)

            # --- per-batch compute ---
            for b in range(BL):
                # energy: E[:, t] = sum_d K[s,:] * q  (fused mul+reduce on DVE)
                E = cb.tile([P, NT + 1], F32, tag=f"E{b}")
                for g in range(NG):
                    for j in range(GJ):
                        t = GJ * g + j
                        tmp = tp.tile([P, D], F32, tag="tmp")
                        nc.vector.affine_mul_reduce(
                            out=tmp[:], accum_out=E[:, t:t + 1],
                            in0=ktiles[b][g][:, j, :], in1=qb[b][:],
                            scale=1.0, bias=0.0)

                # col 16 <- per-partition max over the 16 energy columns
                nc.vector.reduce_max(E[:, NT:NT + 1], E[:, 0:NT],
                                     axis=nc_axis_x)

                # transpose to [16, 128]: row t holds s in [128t, 128t+128)
                Et_ps = pp.tile([NT, P], F32, tag="Et_ps")
                nc.tensor.transpose(Et_ps[:], E[:, 0:NT], ident[:])
                Et = wp.tile([NT, P], F32, tag="Et")
                nc.scalar.copy(Et[:], Et_ps[:])

                # per-partition maxes -> one row, then global max (negated)
                Em_ps = pp.tile([1, P], F32, tag="Em_ps")
                nc.tensor.transpose(Em_ps[:], E[:, NT:NT + 1], ident[:])
                Em = wp.tile([1, P], F32, tag="Em")
                nc.scalar.copy(Em[:], Em_ps[:])
                nm1 = wp.tile([1, 1], F32, tag="nm1")
                nc.vector.tensor_reduce(nm1[:], Em[:],
                                        axis=nc_axis_x, op=alu.max, negate=True)
                nm_ps = pp.tile([NT, 1], F32, tag="nm_ps")
                nc.tensor.matmul(nm_ps[:], ones[0:1, 0:NT], nm1[:],
                                 start=True, stop=True)
                nm = wp.tile([NT, 1], F32, tag="nm")
                nc.scalar.copy(nm[:], nm_ps[:])

                # exp(E - max) with fused row sums -> scat[:,0] (=Z rows)
                X = wp.tile([NT, P], F32, tag="X")
                scat = wp.tile([NT, 2], F32, tag="scat")
                nc.scalar.activation(X[:], Et[:], act_exp,
                                     bias=nm[:], scale=1.0,
                                     accum_out=scat[:, 0:1])

                # mask + fused masked row sums -> scat[:,1]
                W1 = wp.tile([NT, P], F32, tag="W1")
                nc.vector.scalar_tensor_tensor(
                    out=W1[:], in0=iota[:], scalar=th[:, b:b + 1],
                    in1=X[:], op0=alu.is_lt, op1=alu.mult,
                    accum_out=scat[:, 1:2])

                # batch sums: [1,2] = {Z, S_m}
                ss_ps = pp.tile([1, 2], F32, tag="ss_ps")
                nc.tensor.matmul(ss_ps[:], ones[0:NT, 0:1], scat[:],
                                 start=True, stop=True)
                ss = wp.tile([1, 2], F32, tag="ss")
                nc.scalar.copy(ss[:], ss_ps[:])

                # denom = max(S_m, eps*Z); r = 1/denom
                d1 = wp.tile([1, 1], F32, tag="d1")
                nc.vector.tensor_scalar_mul(d1[:], ss[:, 0:1], EPS)
                d2 = wp.tile([1, 1], F32, tag="d2")
                nc.vector.tensor_scalar_max(d2[:], ss[:, 1:2], d1[:])
                r1 = wp.tile([1, 1], F32, tag="r1")
                nc.vector.reciprocal(r1[:], d2[:])
                r_ps = pp.tile([NT, 1], F32, tag="r_ps")
                nc.tensor.matmul(r_ps[:], ones[0:1, 0:NT], r1[:],
                                 start=True, stop=True)
                r16 = wp.tile([NT, 1], F32, tag="r16")
                nc.scalar.copy(r16[:], r_ps[:])

                # final weights
                W = wp.tile([NT, P], F32, tag="W")
                nc.vector.tensor_scalar_mul(W[:], W1[:], r16[:])
                nc.gpsimd.dma_start(w_out[b], W[:])

                # transpose weights back to [128, 16] for the context matmuls
                wT_ps = pp.tile([P, NT], F32, tag="wT_ps")
                nc.tensor.transpose(wT_ps[:], W[:], ident[0:NT, 0:NT])
                wT = wp.tile([P, NT], F32, tag="wT")
                nc.scalar.copy(wT[:], wT_ps[:])

                # context: ctx[1,512] += w_tile.T @ V_tile  (fp32)
                ctx_ps = pp.tile([1, D], F32, tag="ctx_ps")
                for g in range(NG):
                    for j in range(GJ):
                        t = GJ * g + j
                        nc.tensor.matmul(ctx_ps[:], wT[:, t:t + 1],
                                         vtiles[b][g][:, j, :],
                                         start=(t == 0), stop=(t == NT - 1))
                cs = wp.tile([1, D], F32, tag="cs")
                nc.scalar.copy(cs[:], ctx_ps[:])
                nc.gpsimd.dma_start(ctx_out[b:b + 1, :], cs[:])

    nc.compile()
    return nc


def _get_nc():
    if "nc" not in _CACHE:
        from concourse import mybir
        global nc_axis_x, alu, act_exp
        nc_axis_x = mybir.AxisListType.X
        alu = mybir.AluOpType
        act_exp = mybir.ActivationFunctionType.Exp
        _CACHE["nc"] = _build()
    return _CACHE["nc"]


def _maybe_register_trace_hook():
    """run_bass_kernel_spmd(trace=True) under axon needs antenv.axon_hooks;
    the RL image lacks it, so synthesize the module from trn_agent_boot."""
    import sys, types
    if "antenv.axon_hooks" in sys.modules:
        return
    try:
        import trn_agent_boot.trn_boot as tb
        hook = tb._ntff_profile_via_ctypes('/opt/axon/libaxon_pjrt.so')
        mod = types.ModuleType('antenv.axon_hooks')
        mod.get_axon_ntff_profile_hook = lambda: hook
        mod.set_axon_ntff_profile_hook = lambda h: None
        sys.modules['antenv.axon_hooks'] = mod
    except Exception:
        pass


def kernel(key, value, query, encoder_condensed_lens):
    global LAST_RESULT
    key = np.asarray(key, dtype=np.float32)
    value = np.asarray(value, dtype=np.float32)
    query = np.asarray(query, dtype=np.float32)
    lens = np.asarray(encoder_condensed_lens).astype(np.int64).reshape(B)

    iota_np = np.tile(np.arange(P, dtype=np.float32)[None, :], (NT, 1))
    ident_np = np.eye(P, dtype=np.float32)

    in_maps = []
    for c in range(NCORES):
        b0 = c * BL
        th = np.empty((NT, BL), dtype=np.float32)
        for b in range(BL):
            th[:, b] = lens[b0 + b] - P * np.arange(NT)
        in_maps.append({
            "K_in": np.ascontiguousarray(key[:, b0:b0 + BL, :]),
            "V_in": np.ascontiguousarray(value[:, b0:b0 + BL, :]),
            "q_in": np.ascontiguousarray(query[b0:b0 + BL, :]).reshape(1, BL * D),
            "th_in": th,
            "io_in": iota_np,
            "id_in": ident_np,
        })

    nc = _get_nc()
    from concourse import bass_utils
    trace = bool(os.environ.get("BASS_TRACE"))
    if trace:
        _maybe_register_trace_hook()
    res = bass_utils.run_bass_kernel_spmd(
        nc, in_maps, core_ids=list(range(NCORES)), trace=trace)
    LAST_RESULT = res

    context = np.concatenate([r["ctx_out"] for r in res.results], axis=0)
    w = np.concatenate([r["w_out"].reshape(BL, S) for r in res.results], axis=0)
    return context.astype(np.float32), w.astype(np.float32)


# revision 7
# speedup vs baseline: 1.0288x; 1.0288x over previous
"""Trainium2 Bass kernel for ragged-sequence attention.

reference computation (per batch b):
    energy[s] = sum_d key[s,b,:] . query[b,:]          (S=2048, D=512)
    w = softmax(energy) * mask;  w /= max(sum(w), eps)
    context = sum_s w[s] * value[s,b,:]
returns (context (B,D), w (B,S)).

Sharding: pure data parallel over batch, 4 batches per core on 8 cores.

Per-core dataflow:
  - K and V stream in as 4 MiB quad-batch tiles [128p, 4j, 4b, 512d]
    (8 KiB contiguous runs keep HWDGE descriptor generation cheap).
  - energy via fused multiply+reduce (affine_mul_reduce) on DVE.
  - softmax in a transposed [16,128] layout: PE transpose, ACT exp with
    fused row sums, fused mask+multiply+sum, exact eps-clamp semantics
    w = exp*m / max(S_m, eps*Z).
  - context: fp32 skinny matmuls packed 4-wide into PE column groups
    via tile_position, accumulating over the 16 s-tiles.
"""
import contextlib
import os

import numpy as np

S, B, D = 2048, 32, 512
NCORES = 8
BL = B // NCORES          # 4 local batches per core
P = 128                   # partitions
NT = S // P               # 16 s-tiles of 128
NG = 4                    # 4 quad-batch groups: tile = 4 s-tiles x 4 batches
GJ = NT // NG             # s-tiles per group
EPS = 1e-12

_CACHE = {}
LAST_RESULT = None


def _build():
    from concourse import bacc, mybir
    import concourse.tile as tile

    F32 = mybir.dt.float32
    AX = mybir.AxisListType.X
    ALU = mybir.AluOpType
    EXP = mybir.ActivationFunctionType.Exp

    nc = bacc.Bacc("TRN2", target_bir_lowering=False, debug=False,
                   num_devices=NCORES)

    K_in = nc.dram_tensor("K_in", [S, BL, D], F32, kind="ExternalInput").ap()
    V_in = nc.dram_tensor("V_in", [S, BL, D], F32, kind="ExternalInput").ap()
    q_in = nc.dram_tensor("q_in", [1, BL * D], F32, kind="ExternalInput").ap()
    th_in = nc.dram_tensor("th_in", [NT, BL], F32, kind="ExternalInput").ap()
    io_in = nc.dram_tensor("io_in", [NT, P], F32, kind="ExternalInput").ap()
    id_in = nc.dram_tensor("id_in", [P, P], F32, kind="ExternalInput").ap()
    ctx_out = nc.dram_tensor("ctx_out", [BL, D], F32, kind="ExternalOutput").ap()
    w_out = nc.dram_tensor("w_out", [BL, NT, P], F32, kind="ExternalOutput").ap()

    # (g j p) b d -> g p j b d : quad tile g = [128p, 4j, 4b, 512d] (4 MiB)
    Kv = K_in.rearrange("(g j p) b d -> g p j b d", g=NG, j=GJ, p=P)
    Vv = V_in.rearrange("(g j p) b d -> g p j b d", g=NG, j=GJ, p=P)

    with tile.TileContext(nc) as tc:
        with contextlib.ExitStack() as ctx:
            kp = ctx.enter_context(tc.tile_pool(name="kp", bufs=2))
            vp = ctx.enter_context(tc.tile_pool(name="vp", bufs=3))
            cb = ctx.enter_context(tc.tile_pool(name="cb", bufs=1))
            tp = ctx.enter_context(tc.tile_pool(name="tp", bufs=2))
            wp = ctx.enter_context(tc.tile_pool(name="wp", bufs=2))
            pp = ctx.enter_context(tc.tile_pool(name="pp", bufs=1, space="PSUM"))

            # --- constants first on the sync ring (tiny) ---
            ident = cb.tile([P, P], F32)
            nc.sync.dma_start(ident[:], id_in[:])
            iota = cb.tile([NT, P], F32)
            nc.sync.dma_start(iota[:], io_in[:])
            th = cb.tile([NT, BL], F32)
            nc.sync.dma_start(th[:], th_in[:])
            qsb = cb.tile([1, BL * D], F32)
            nc.sync.dma_start(qsb[:], q_in[:])
            ones = cb.tile([NT, P], F32)
            nc.gpsimd.memset(ones[:], 1.0)

            # --- big-stream DMAs: K quads then V quads, one HWDGE ring ---
            ktiles, vtiles = [], []
            for g in range(NG):
                t = kp.tile([P, GJ, BL, D], F32, tag="kt")
                nc.sync.dma_start(t[:], Kv[g])
                ktiles.append(t)
            for g in range(NG):
                t = vp.tile([P, GJ, BL, D], F32, tag="vt")
                nc.sync.dma_start(t[:], Vv[g])
                vtiles.append(t)

            # --- q broadcast to [128, 512] per batch (PE) ---
            qb = []
            for b in range(BL):
                qb_ps = pp.tile([P, D], F32, tag="qb_ps")
                nc.tensor.matmul(qb_ps[:], ones[0:1, :],
                                 qsb[0:1, b * D:(b + 1) * D],
                                 start=True, stop=True)
                qb_b = cb.tile([P, D], F32, tag=f"qb{b}")
                nc.scalar.copy(qb_b[:], qb_ps[:])
                qb.append(qb_b)

            # --- energy: E_b[:, t] = sum_d K[s,:]*q (fused DVE), K-paced ---
            Es = [cb.tile([P, NT + 1], F32, tag=f"E{b}", name=f"E{b}")
                  for b in range(BL)]
            for g in range(NG):
                for j in range(GJ):
                    for b in range(BL):
                        t = GJ * g + j
                        tmp = tp.tile([P, D], F32, tag="tmp")
                        nc.vector.affine_mul_reduce(
                            out=tmp[:], accum_out=Es[b][:, t:t + 1],
                            in0=ktiles[g][:, j, b, :], in1=qb[b][:],
                            scale=1.0, bias=0.0)

            # --- per-batch softmax; wT_b = [128, 16] weight columns ---
            wTs = []
            for b in range(BL):
                E = Es[b]
                nc.vector.reduce_max(E[:, NT:NT + 1], E[:, 0:NT], axis=AX)

                Et_ps = pp.tile([NT, P], F32, tag="Et_ps")
                nc.tensor.transpose(Et_ps[:], E[:, 0:NT], ident[:])
                Et = wp.tile([NT, P], F32, tag="Et")
                nc.scalar.copy(Et[:], Et_ps[:])

                Em_ps = pp.tile([1, P], F32, tag="Em_ps")
                nc.tensor.transpose(Em_ps[:], E[:, NT:NT + 1], ident[:])
                Em = wp.tile([1, P], F32, tag="Em")
                nc.scalar.copy(Em[:], Em_ps[:])
                nm1 = wp.tile([1, 1], F32, tag="nm1")
                nc.vector.tensor_reduce(nm1[:], Em[:], axis=AX, op=ALU.max,
                                        negate=True)
                nm_ps = pp.tile([NT, 1], F32, tag="nm_ps")
                nc.tensor.matmul(nm_ps[:], ones[0:1, 0:NT], nm1[:],
                                 start=True, stop=True)
                nm = wp.tile([NT, 1], F32, tag="nm")
                nc.scalar.copy(nm[:], nm_ps[:])

                X = wp.tile([NT, P], F32, tag="X")
                scat = wp.tile([NT, 2], F32, tag="scat")
                nc.scalar.activation(X[:], Et[:], EXP, bias=nm[:], scale=1.0,
                                     accum_out=scat[:, 0:1])

                W1 = wp.tile([NT, P], F32, tag="W1")
                nc.vector.scalar_tensor_tensor(
                    out=W1[:], in0=iota[:], scalar=th[:, b:b + 1], in1=X[:],
                    op0=ALU.is_lt, op1=ALU.mult, accum_out=scat[:, 1:2])

                ss_ps = pp.tile([1, 2], F32, tag="ss_ps")
                nc.tensor.matmul(ss_ps[:], ones[0:NT, 0:1], scat[:],
                                 start=True, stop=True)
                ss = wp.tile([1, 2], F32, tag="ss")
                nc.scalar.copy(ss[:], ss_ps[:])

                d1 = wp.tile([1, 1], F32, tag="d1")
                nc.vector.tensor_scalar_mul(d1[:], ss[:, 0:1], EPS)
                d2 = wp.tile([1, 1], F32, tag="d2")
                nc.vector.tensor_scalar_max(d2[:], ss[:, 1:2], d1[:])
                r1 = wp.tile([1, 1], F32, tag="r1")
                nc.vector.reciprocal(r1[:], d2[:])
                r_ps = pp.tile([NT, 1], F32, tag="r_ps")
                nc.tensor.matmul(r_ps[:], ones[0:1, 0:NT], r1[:],
                                 start=True, stop=True)
                r16 = wp.tile([NT, 1], F32, tag="r16")
                nc.scalar.copy(r16[:], r_ps[:])

                W = wp.tile([NT, P], F32, tag=f"W{b}")
                nc.vector.tensor_scalar_mul(W[:], W1[:], r16[:])
                nc.sync.dma_start(w_out[b], W[:])

                wT_ps = pp.tile([P, NT], F32, tag="wT_ps")
                nc.tensor.transpose(wT_ps[:], W[:], ident[0:NT, 0:NT])
                wT = wp.tile([P, NT], F32, tag=f"wT{b}")
                nc.scalar.copy(wT[:], wT_ps[:])
                wTs.append(wT)

            # --- context: 4 batches packed into PE column groups ---
            ctx_ps = pp.tile([P, D], F32, tag="ctx_ps")
            for g in range(NG):
                for j in range(GJ):
                    t = GJ * g + j
                    for b in range(BL):
                        nc.tensor.matmul(
                            ctx_ps[32 * b:32 * b + 1, :],
                            wTs[b][:, t:t + 1], vtiles[g][:, j, b, :],
                            start=(t == 0), stop=(t == NT - 1),
                            tile_position=(0, 32 * b))
            for b in range(BL):
                cs = wp.tile([1, D], F32, tag=f"cs{b}")
                nc.scalar.copy(cs[:], ctx_ps[32 * b:32 * b + 1, :])
                nc.sync.dma_start(ctx_out[b:b + 1, :], cs[:])

    nc.compile()
    return nc


def _get_nc():
    if "nc" not in _CACHE:
        _CACHE["nc"] = _build()
    return _CACHE["nc"]


def _maybe_register_trace_hook():
    """run_bass_kernel_spmd(trace=True) under axon needs antenv.axon_hooks;
    the RL image lacks it, so synthesize the module from trn_agent_boot."""
    import sys, types
    if "antenv.axon_hooks" in sys.modules:
        return
    try:
        import trn_agent_boot.trn_boot as tb
        hook = tb._ntff_profile_via_ctypes('/opt/axon/libaxon_pjrt.so')
        mod = types.ModuleType('antenv.axon_hooks')
        mod.get_axon_ntff_profile_hook = lambda: hook
        mod.set_axon_ntff_profile_hook = lambda h: None
        sys.modules['antenv.axon_hooks'] = mod
    except Exception:
        pass


def kernel(key, value, query, encoder_condensed_lens):
    global LAST_RESULT
    key = np.asarray(key, dtype=np.float32)
    value = np.asarray(value, dtype=np.float32)
    query = np.asarray(query, dtype=np.float32)
    lens = np.asarray(encoder_condensed_lens).astype(np.int64).reshape(B)

    iota_np = np.tile(np.arange(P, dtype=np.float32)[None, :], (NT, 1))
    ident_np = np.eye(P, dtype=np.float32)

    in_maps = []
    for c in range(NCORES):
        b0 = c * BL
        th = np.empty((NT, BL), dtype=np.float32)
        for b in range(BL):
            th[:, b] = lens[b0 + b] - P * np.arange(NT)
        in_maps.append({
            "K_in": np.ascontiguousarray(key[:, b0:b0 + BL, :]),
            "V_in": np.ascontiguousarray(value[:, b0:b0 + BL, :]),
            "q_in": np.ascontiguousarray(query[b0:b0 + BL, :]).reshape(1, BL * D),
            "th_in": th,
            "io_in": iota_np,
            "id_in": ident_np,
        })

    nc = _get_nc()
    from concourse import bass_utils
    trace = bool(os.environ.get("BASS_TRACE"))
    if trace:
        _maybe_register_trace_hook()
    res = bass_utils.run_bass_kernel_spmd(
        nc, in_maps, core_ids=list(range(NCORES)), trace=trace)
    LAST_RESULT = res

    context = np.concatenate([r["ctx_out"] for r in res.results], axis=0)
    w = np.concatenate([r["w_out"].reshape(BL, S) for r in res.results], axis=0)
    return context.astype(np.float32), w.astype(np.float32)


# revision 10
# speedup vs baseline: 1.0356x; 1.0066x over previous
"""Trainium2 Bass kernel for ragged-sequence attention.

reference computation (per batch b):
    energy[s] = sum_d key[s,b,:] . query[b,:]          (S=2048, D=512)
    w = softmax(energy) * mask;  w /= max(sum(w), eps)
    context = sum_s w[s] * value[s,b,:]
returns (context (B,D), w (B,S)).

Sharding: pure data parallel over batch, 4 batches per core on 8 cores.

Per-core dataflow:
  - K and V stream in as 4 MiB quad-batch tiles [128p, 4j, 4b, 512d]
    (8 KiB contiguous runs keep HWDGE descriptor generation cheap).
  - energy via fused multiply+reduce (affine_mul_reduce) on DVE.
  - softmax in a transposed [16,128] layout: PE transpose, ACT exp with
    fused row sums, fused mask+multiply+sum, exact eps-clamp semantics
    w = exp*m / max(S_m, eps*Z).
  - context: fp32 skinny matmuls packed 4-wide into PE column groups
    via tile_position, accumulating over the 16 s-tiles.
"""
import contextlib
import os

import numpy as np

S, B, D = 2048, 32, 512
NCORES = 8
BL = B // NCORES          # 4 local batches per core
P = 128                   # partitions
NT = S // P               # 16 s-tiles of 128
NG = 16                   # quad-batch tiles: one s-tile x 4 batches (1 MiB)
EPS = 1e-12

_CACHE = {}
LAST_RESULT = None


def _build():
    from concourse import bacc, mybir
    import concourse.tile as tile

    F32 = mybir.dt.float32
    AX = mybir.AxisListType.X
    ALU = mybir.AluOpType
    EXP = mybir.ActivationFunctionType.Exp

    nc = bacc.Bacc("TRN2", target_bir_lowering=False, debug=False,
                   num_devices=NCORES)

    K_in = nc.dram_tensor("K_in", [S, BL, D], F32, kind="ExternalInput").ap()
    V_in = nc.dram_tensor("V_in", [S, BL, D], F32, kind="ExternalInput").ap()
    q_in = nc.dram_tensor("q_in", [1, BL * D], F32, kind="ExternalInput").ap()
    th_in = nc.dram_tensor("th_in", [NT, BL], F32, kind="ExternalInput").ap()
    io_in = nc.dram_tensor("io_in", [NT, P], F32, kind="ExternalInput").ap()
    id_in = nc.dram_tensor("id_in", [P, P], F32, kind="ExternalInput").ap()
    ctx_out = nc.dram_tensor("ctx_out", [BL, D], F32, kind="ExternalOutput").ap()
    w_out = nc.dram_tensor("w_out", [BL, NT, P], F32, kind="ExternalOutput").ap()

    # (t p) b d -> t p b d : quad tile t = [128p, 4b, 512d] (1 MiB)
    Kv = K_in.rearrange("(t p) b d -> t p b d", t=NT, p=P)
    Vv = V_in.rearrange("(t p) b d -> t p b d", t=NT, p=P)

    with tile.TileContext(nc) as tc:
        with contextlib.ExitStack() as ctx:
            kp = ctx.enter_context(tc.tile_pool(name="kp", bufs=6))
            vp = ctx.enter_context(tc.tile_pool(name="vp", bufs=8))
            cb = ctx.enter_context(tc.tile_pool(name="cb", bufs=1))
            tp = ctx.enter_context(tc.tile_pool(name="tp", bufs=2))
            wp = ctx.enter_context(tc.tile_pool(name="wp", bufs=2))
            pp = ctx.enter_context(tc.tile_pool(name="pp", bufs=1, space="PSUM"))

            # --- constants first on the sync ring (tiny) ---
            ident = cb.tile([P, P], F32)
            nc.sync.dma_start(ident[:], id_in[:])
            iota = cb.tile([NT, P], F32)
            nc.sync.dma_start(iota[:], io_in[:])
            th = cb.tile([NT, BL], F32)
            nc.sync.dma_start(th[:], th_in[:])
            qsb = cb.tile([1, BL * D], F32)
            nc.sync.dma_start(qsb[:], q_in[:])
            ones = cb.tile([NT, P], F32)
            nc.gpsimd.memset(ones[:], 1.0)

            # --- big-stream DMAs: K quads then V quads, one HWDGE ring ---
            ktiles, vtiles = [], []
            for t_i in range(NT):
                t = kp.tile([P, BL, D], F32, tag="kt")
                nc.sync.dma_start(t[:], Kv[t_i])
                ktiles.append(t)
            for t_i in range(NT):
                t = vp.tile([P, BL, D], F32, tag="vt")
                nc.sync.dma_start(t[:], Vv[t_i])
                vtiles.append(t)

            # --- q broadcast to [128, 512] per batch (PE) ---
            qb = []
            for b in range(BL):
                qb_ps = pp.tile([P, D], F32, tag="qb_ps")
                nc.tensor.matmul(qb_ps[:], ones[0:1, :],
                                 qsb[0:1, b * D:(b + 1) * D],
                                 start=True, stop=True)
                qb_b = cb.tile([P, D], F32, tag=f"qb{b}")
                nc.scalar.copy(qb_b[:], qb_ps[:])
                qb.append(qb_b)

            # --- energy: E_b[:, t] = sum_d K[s,:]*q (fused DVE), K-paced ---
            Es = [cb.tile([P, NT + 1], F32, tag=f"E{b}", name=f"E{b}")
                  for b in range(BL)]
            for t in range(NT):
                for b in range(BL):
                    tmp = tp.tile([P, D], F32, tag="tmp")
                    nc.vector.affine_mul_reduce(
                        out=tmp[:], accum_out=Es[b][:, t:t + 1],
                        in0=ktiles[t][:, b, :], in1=qb[b][:],
                        scale=1.0, bias=0.0)

            # --- per-batch softmax; wT_b = [128, 16] weight columns ---
            wTs = []
            for b in range(BL):
                E = Es[b]
                nc.vector.reduce_max(E[:, NT:NT + 1], E[:, 0:NT], axis=AX)

                Et_ps = pp.tile([NT, P], F32, tag="Et_ps")
                nc.tensor.transpose(Et_ps[:], E[:, 0:NT], ident[:])
                Et = wp.tile([NT, P], F32, tag="Et")
                nc.scalar.copy(Et[:], Et_ps[:])

                Em_ps = pp.tile([1, P], F32, tag="Em_ps")
                nc.tensor.transpose(Em_ps[:], E[:, NT:NT + 1], ident[:])
                Em = wp.tile([1, P], F32, tag="Em")
                nc.scalar.copy(Em[:], Em_ps[:])
                nm1 = wp.tile([1, 1], F32, tag="nm1")
                nc.vector.tensor_reduce(nm1[:], Em[:], axis=AX, op=ALU.max,
                                        negate=True)
                nm_ps = pp.tile([NT, 1], F32, tag="nm_ps")
                nc.tensor.matmul(nm_ps[:], ones[0:1, 0:NT], nm1[:],
                                 start=True, stop=True)
                nm = wp.tile([NT, 1], F32, tag="nm")
                nc.scalar.copy(nm[:], nm_ps[:])

                X = wp.tile([NT, P], F32, tag="X")
                scat = wp.tile([NT, 2], F32, tag="scat")
                nc.scalar.activation(X[:], Et[:], EXP, bias=nm[:], scale=1.0,
                                     accum_out=scat[:, 0:1])

                W1 = wp.tile([NT, P], F32, tag="W1")
                nc.vector.scalar_tensor_tensor(
                    out=W1[:], in0=iota[:], scalar=th[:, b:b + 1], in1=X[:],
                    op0=ALU.is_lt, op1=ALU.mult, accum_out=scat[:, 1:2])

                ss_ps = pp.tile([1, 2], F32, tag="ss_ps")
                nc.tensor.matmul(ss_ps[:], ones[0:NT, 0:1], scat[:],
                                 start=True, stop=True)
                ss = wp.tile([1, 2], F32, tag="ss")
                nc.scalar.copy(ss[:], ss_ps[:])

                d1 = wp.tile([1, 1], F32, tag="d1")
                nc.vector.tensor_scalar_mul(d1[:], ss[:, 0:1], EPS)
                d2 = wp.tile([1, 1], F32, tag="d2")
                nc.vector.tensor_scalar_max(d2[:], ss[:, 1:2], d1[:])
                r1 = wp.tile([1, 1], F32, tag="r1")
                nc.vector.reciprocal(r1[:], d2[:])
                r_ps = pp.tile([NT, 1], F32, tag="r_ps")
                nc.tensor.matmul(r_ps[:], ones[0:1, 0:NT], r1[:],
                                 start=True, stop=True)
                r16 = wp.tile([NT, 1], F32, tag="r16")
                nc.scalar.copy(r16[:], r_ps[:])

                W = wp.tile([NT, P], F32, tag=f"W{b}")
                nc.vector.tensor_scalar_mul(W[:], W1[:], r16[:])
                nc.gpsimd.dma_start(w_out[b], W[:])

                wT_ps = pp.tile([P, NT], F32, tag="wT_ps")
                nc.tensor.transpose(wT_ps[:], W[:], ident[0:NT, 0:NT])
                wT = wp.tile([P, NT], F32, tag=f"wT{b}")
                nc.scalar.copy(wT[:], wT_ps[:])
                wTs.append(wT)

            # --- context: 4 batches packed into PE column groups ---
            ctx_ps = pp.tile([P, D], F32, tag="ctx_ps")
            for t in range(NT):
                for b in range(BL):
                    nc.tensor.matmul(
                        ctx_ps[32 * b:32 * b + 1, :],
                        wTs[b][:, t:t + 1], vtiles[t][:, b, :],
                        start=(t == 0), stop=(t == NT - 1),
                        tile_position=(0, 32 * b))
            for b in range(BL):
                cs = wp.tile([1, D], F32, tag=f"cs{b}", name=f"cs{b}")
                if b % 2 == 0:
                    nc.scalar.copy(cs[:], ctx_ps[32 * b:32 * b + 1, :])
                else:
                    nc.vector.tensor_copy(cs[:], ctx_ps[32 * b:32 * b + 1, :])
                nc.gpsimd.dma_start(ctx_out[b:b + 1, :], cs[:])

    nc.compile()
    return nc


def _get_nc():
    if "nc" not in _CACHE:
        _CACHE["nc"] = _build()
    return _CACHE["nc"]


def _maybe_register_trace_hook():
    """run_bass_kernel_spmd(trace=True) under axon needs antenv.axon_hooks;
    the RL image lacks it, so synthesize the module from trn_agent_boot."""
    import sys, types
    if "antenv.axon_hooks" in sys.modules:
        return
    try:
        import trn_agent_boot.trn_boot as tb
        hook = tb._ntff_profile_via_ctypes('/opt/axon/libaxon_pjrt.so')
        mod = types.ModuleType('antenv.axon_hooks')
        mod.get_axon_ntff_profile_hook = lambda: hook
        mod.set_axon_ntff_profile_hook = lambda h: None
        sys.modules['antenv.axon_hooks'] = mod
    except Exception:
        pass


def kernel(key, value, query, encoder_condensed_lens):
    global LAST_RESULT
    key = np.asarray(key, dtype=np.float32)
    value = np.asarray(value, dtype=np.float32)
    query = np.asarray(query, dtype=np.float32)
    lens = np.asarray(encoder_condensed_lens).astype(np.int64).reshape(B)

    iota_np = np.tile(np.arange(P, dtype=np.float32)[None, :], (NT, 1))
    ident_np = np.eye(P, dtype=np.float32)

    in_maps = []
    for c in range(NCORES):
        b0 = c * BL
        th = np.empty((NT, BL), dtype=np.float32)
        for b in range(BL):
            th[:, b] = lens[b0 + b] - P * np.arange(NT)
        in_maps.append({
            "K_in": np.ascontiguousarray(key[:, b0:b0 + BL, :]),
            "V_in": np.ascontiguousarray(value[:, b0:b0 + BL, :]),
            "q_in": np.ascontiguousarray(query[b0:b0 + BL, :]).reshape(1, BL * D),
            "th_in": th,
            "io_in": iota_np,
            "id_in": ident_np,
        })

    nc = _get_nc()
    from concourse import bass_utils
    trace = bool(os.environ.get("BASS_TRACE"))
    if trace:
        _maybe_register_trace_hook()
    res = bass_utils.run_bass_kernel_spmd(
        nc, in_maps, core_ids=list(range(NCORES)), trace=trace)
    LAST_RESULT = res

    context = np.concatenate([r["ctx_out"] for r in res.results], axis=0)
    w = np.concatenate([r["w_out"].reshape(BL, S) for r in res.results], axis=0)
    return context.astype(np.float32), w.astype(np.float32)


# revision 11
# speedup vs baseline: 1.2715x; 1.2278x over previous
"""Trainium2 Bass kernel for ragged-sequence attention.

reference computation (per batch b):
    energy[s] = sum_d key[s,b,:] . query[b,:]          (S=2048, D=512)
    w = softmax(energy) * mask;  w /= max(sum(w), eps)
    context = sum_s w[s] * value[s,b,:]
returns (context (B,D), w (B,S)).

Sharding: data parallel over batch, 4 batches per core on 8 cores, with a
rank-balanced batch->core assignment that equalizes ragged V traffic.

Raggedness: w is exactly zero for s >= len_b, so V rows beyond
ceil(len_b/128)*128 never contribute to the context.  Batches are sorted by
tile count and assigned round-robin (core c gets ranks {c, 8+c, 16+c, 24+c}),
so one compiled program with per-slot V tile counts Q = (TB[rank 0],
TB[rank 8], TB[rank 16], TB[rank 24]) serves every core.  The program is
compiled per Q-profile and cached.

Per-core dataflow:
  - K streams in as 1 MiB quad-batch tiles [128p, 4slot, 512d] (8 KiB runs).
  - energy via fused multiply+reduce (affine_mul_reduce) on DVE against a
    gpsimd partition-broadcast of q.
  - softmax in a transposed [16,128] layout: PE transpose, ACT exp with
    fused row sums, fused mask+multiply+sum, eps-clamp semantics
    w = exp*m / max(S_m, eps*Z)  (Z = full-row sum, matching jax.nn.softmax
    + renormalize exactly, including the underflow-clamp regime).
  - context: fp32 skinny matmuls packed up to 4-wide into PE column groups
    via tile_position, only over the ragged V tiles.
"""
import contextlib
import os

import numpy as np

S, B, D = 2048, 32, 512
NCORES = 8
BL = B // NCORES          # 4 local batches (slots) per core
P = 128                   # partitions
NT = S // P               # 16 s-tiles of 128
EPS = 1e-12

_CACHE = {}
LAST_RESULT = None


def _build(Q):
    from concourse import bacc, mybir
    import concourse.tile as tile

    F32 = mybir.dt.float32
    AX = mybir.AxisListType.X
    ALU = mybir.AluOpType
    EXP = mybir.ActivationFunctionType.Exp
    NV = sum(Q)

    nc = bacc.Bacc("TRN2", target_bir_lowering=False, debug=False,
                   num_devices=NCORES)

    K_in = nc.dram_tensor("K_in", [S, BL, D], F32, kind="ExternalInput").ap()
    V_in = nc.dram_tensor("V_in", [S, BL, D], F32, kind="ExternalInput").ap()
    q_in = nc.dram_tensor("q_in", [1, BL * D], F32, kind="ExternalInput").ap()
    th_in = nc.dram_tensor("th_in", [NT, BL], F32, kind="ExternalInput").ap()
    io_in = nc.dram_tensor("io_in", [NT, P], F32, kind="ExternalInput").ap()
    id_in = nc.dram_tensor("id_in", [P, P], F32, kind="ExternalInput").ap()
    ctx_out = nc.dram_tensor("ctx_out", [BL, D], F32, kind="ExternalOutput").ap()
    w_out = nc.dram_tensor("w_out", [BL, NT, P], F32, kind="ExternalOutput").ap()

    # K quad tile t = [128p, 4slot, 512d] (1 MiB); V per-slot tile [128p, 512d]
    Kv = K_in.rearrange("(t p) b d -> t p b d", t=NT, p=P)
    Vv = V_in.rearrange("(t p) b d -> t b p d", t=NT, p=P)

    with tile.TileContext(nc) as tc:
        with contextlib.ExitStack() as ctx:
            kp = ctx.enter_context(tc.tile_pool(name="kp", bufs=6))
            vp = ctx.enter_context(tc.tile_pool(name="vp", bufs=NV))
            cb = ctx.enter_context(tc.tile_pool(name="cb", bufs=1))
            tp = ctx.enter_context(tc.tile_pool(name="tp", bufs=2))
            wp = ctx.enter_context(tc.tile_pool(name="wp", bufs=4))
            pp = ctx.enter_context(tc.tile_pool(name="pp", bufs=1, space="PSUM"))

            # --- constants first on the sync ring (tiny) ---
            ident = cb.tile([P, P], F32)
            nc.sync.dma_start(ident[:], id_in[:])
            iota = cb.tile([NT, P], F32)
            nc.sync.dma_start(iota[:], io_in[:])
            th = cb.tile([NT, BL], F32)
            nc.sync.dma_start(th[:], th_in[:])
            qsb = cb.tile([1, BL * D], F32)
            nc.sync.dma_start(qsb[:], q_in[:])
            ones = cb.tile([NT, P], F32)
            nc.gpsimd.memset(ones[:], 1.0)

            # --- big-stream DMAs: K quads then ragged V, one HWDGE ring ---
            ktiles = []
            for t_i in range(NT):
                t = kp.tile([P, BL, D], F32, tag="kt")
                nc.sync.dma_start(t[:], Kv[t_i])
                ktiles.append(t)
            vtiles = {}
            for t_i in range(NT):
                for s in range(BL):
                    if t_i < Q[s]:
                        t = vp.tile([P, D], F32, tag="vt")
                        nc.sync.dma_start(t[:], Vv[t_i, s])
                        vtiles[(s, t_i)] = t

            # --- q broadcast to [128, 512] per slot (gpsimd) ---
            qb = []
            for b in range(BL):
                qb_b = cb.tile([P, D], F32, tag=f"qb{b}", name=f"qb{b}")
                nc.gpsimd.partition_broadcast(
                    qb_b[:], qsb[0:1, b * D:(b + 1) * D])
                qb.append(qb_b)

            # --- energy: E_b[:, t] = sum_d K[s,:]*q (fused DVE), K-paced ---
            Es = [cb.tile([P, NT + 1], F32, tag=f"E{b}", name=f"E{b}")
                  for b in range(BL)]
            for t in range(NT):
                for b in range(BL):
                    tmp = tp.tile([P, D], F32, tag="tmp")
                    nc.vector.affine_mul_reduce(
                        out=tmp[:], accum_out=Es[b][:, t:t + 1],
                        in0=ktiles[t][:, b, :], in1=qb[b][:],
                        scale=1.0, bias=0.0)

            # --- per-slot softmax; wT_b = [128, 16] weight columns ---
            wTs = []
            for b in range(BL):
                E = Es[b]
                nc.vector.reduce_max(E[:, NT:NT + 1], E[:, 0:NT], axis=AX)

                Et_ps = pp.tile([NT, P], F32, tag="Et_ps", bufs=2)
                nc.tensor.transpose(Et_ps[:], E[:, 0:NT], ident[:])
                Et = wp.tile([NT, P], F32, tag="Et")
                nc.scalar.copy(Et[:], Et_ps[:])

                Em_ps = pp.tile([1, P], F32, tag="sm_ps", name="Em_ps")
                nc.tensor.transpose(Em_ps[:], E[:, NT:NT + 1], ident[:])
                Em = wp.tile([1, P], F32, tag="Em")
                nc.scalar.copy(Em[:], Em_ps[:])
                nm1 = wp.tile([1, 1], F32, tag="nm1")
                nc.vector.tensor_reduce(nm1[:], Em[:], axis=AX, op=ALU.max,
                                        negate=True)
                nm_ps = pp.tile([NT, 1], F32, tag="sm_ps", name="nm_ps")
                nc.tensor.matmul(nm_ps[:], ones[0:1, 0:NT], nm1[:],
                                 start=True, stop=True)
                nm = wp.tile([NT, 1], F32, tag="nm")
                nc.scalar.copy(nm[:], nm_ps[:])

                X = wp.tile([NT, P], F32, tag="X")
                scat = wp.tile([NT, 2], F32, tag="scat")
                nc.scalar.activation(X[:], Et[:], EXP, bias=nm[:], scale=1.0,
                                     accum_out=scat[:, 0:1])

                W1 = wp.tile([NT, P], F32, tag="W1")
                nc.vector.scalar_tensor_tensor(
                    out=W1[:], in0=iota[:], scalar=th[:, b:b + 1], in1=X[:],
                    op0=ALU.is_lt, op1=ALU.mult, accum_out=scat[:, 1:2])

                ss_ps = pp.tile([1, 2], F32, tag="sm_ps", name="ss_ps")
                nc.tensor.matmul(ss_ps[:], ones[0:NT, 0:1], scat[:],
                                 start=True, stop=True)
                ss = wp.tile([1, 2], F32, tag="ss")
                nc.scalar.copy(ss[:], ss_ps[:])

                d1 = wp.tile([1, 1], F32, tag="d1")
                nc.vector.tensor_scalar_mul(d1[:], ss[:, 0:1], EPS)
                d2 = wp.tile([1, 1], F32, tag="d2")
                nc.vector.tensor_scalar_max(d2[:], ss[:, 1:2], d1[:])
                r1 = wp.tile([1, 1], F32, tag="r1")
                nc.vector.reciprocal(r1[:], d2[:])
                r_ps = pp.tile([NT, 1], F32, tag="sm_ps", name="r_ps")
                nc.tensor.matmul(r_ps[:], ones[0:1, 0:NT], r1[:],
                                 start=True, stop=True)
                r16 = wp.tile([NT, 1], F32, tag="r16")
                nc.scalar.copy(r16[:], r_ps[:])

                W = wp.tile([NT, P], F32, tag=f"W{b}", name=f"W{b}")
                nc.vector.tensor_scalar_mul(W[:], W1[:], r16[:])
                nc.gpsimd.dma_start(w_out[b], W[:])

                wT_ps = pp.tile([P, NT], F32, tag="wT_ps")
                nc.tensor.transpose(wT_ps[:], W[:], ident[0:NT, 0:NT])
                wT = wp.tile([P, NT], F32, tag=f"wT{b}", name=f"wT{b}")
                nc.scalar.copy(wT[:], wT_ps[:])
                wTs.append(wT)

            # --- context: ragged, packed into PE column groups per t ---
            cps = [pp.tile([P, D], F32, tag="ctx_ps", bufs=BL, name=f"cps{b}")
                   for b in range(BL)]
            for t in range(NT):
                for b in range(BL):
                    if t < Q[b]:
                        nc.tensor.matmul(
                            cps[b][32 * b:32 * b + 1, :],
                            wTs[b][:, t:t + 1], vtiles[(b, t)][:],
                            start=(t == 0), stop=(t == Q[b] - 1),
                            tile_position=(0, 32 * b))
            for b in range(BL):
                cs = wp.tile([1, D], F32, tag=f"cs{b}", name=f"cs{b}")
                if b % 2 == 0:
                    nc.scalar.copy(cs[:], cps[b][32 * b:32 * b + 1, :])
                else:
                    nc.vector.tensor_copy(cs[:], cps[b][32 * b:32 * b + 1, :])
                nc.sync.dma_start(ctx_out[b:b + 1, :], cs[:])

    nc.compile()
    return nc


def _get_nc(Q):
    if Q not in _CACHE:
        _CACHE[Q] = _build(Q)
    return _CACHE[Q]


def _maybe_register_trace_hook():
    """run_bass_kernel_spmd(trace=True) under axon needs antenv.axon_hooks;
    the RL image lacks it, so synthesize the module from trn_agent_boot."""
    import sys, types
    if "antenv.axon_hooks" in sys.modules:
        return
    try:
        import trn_agent_boot.trn_boot as tb
        hook = tb._ntff_profile_via_ctypes('/opt/axon/libaxon_pjrt.so')
        mod = types.ModuleType('antenv.axon_hooks')
        mod.get_axon_ntff_profile_hook = lambda: hook
        mod.set_axon_ntff_profile_hook = lambda h: None
        sys.modules['antenv.axon_hooks'] = mod
    except Exception:
        pass


def kernel(key, value, query, encoder_condensed_lens):
    global LAST_RESULT
    key = np.asarray(key, dtype=np.float32)
    value = np.asarray(value, dtype=np.float32)
    query = np.asarray(query, dtype=np.float32)
    lens = np.asarray(encoder_condensed_lens).astype(np.int64).reshape(B)

    # rank-balanced assignment: sort batches by V tile count (desc); core c
    # slot k <- rank 8k+c.  Q[k] = max tile count within slot k = rank 8k.
    TB = np.maximum(np.ceil(lens / P).astype(int), 1)
    order = np.argsort(-TB, kind="stable")
    Q = tuple(int(TB[order[8 * k]]) for k in range(BL))

    iota_np = np.tile(np.arange(P, dtype=np.float32)[None, :], (NT, 1))
    ident_np = np.eye(P, dtype=np.float32)

    in_maps = []
    batch_of = np.empty((NCORES, BL), dtype=int)
    for c in range(NCORES):
        bsel = [int(order[8 * k + c]) for k in range(BL)]
        batch_of[c] = bsel
        th = np.empty((NT, BL), dtype=np.float32)
        for k in range(BL):
            th[:, k] = lens[bsel[k]] - P * np.arange(NT)
        in_maps.append({
            "K_in": np.ascontiguousarray(key[:, bsel, :]),
            "V_in": np.ascontiguousarray(value[:, bsel, :]),
            "q_in": np.ascontiguousarray(query[bsel, :]).reshape(1, BL * D),
            "th_in": th,
            "io_in": iota_np,
            "id_in": ident_np,
        })

    nc = _get_nc(Q)
    from concourse import bass_utils
    trace = bool(os.environ.get("BASS_TRACE"))
    if trace:
        _maybe_register_trace_hook()
    res = bass_utils.run_bass_kernel_spmd(
        nc, in_maps, core_ids=list(range(NCORES)), trace=trace)
    LAST_RESULT = res

    context = np.empty((B, D), dtype=np.float32)
    w = np.empty((B, S), dtype=np.float32)
    for c in range(NCORES):
        r = res.results[c]
        for k in range(BL):
            context[batch_of[c][k]] = r["ctx_out"][k]
            w[batch_of[c][k]] = r["w_out"][k].reshape(S)
    return context, w


# revision 13
# speedup vs baseline: 1.3322x; 1.0477x over previous
"""Trainium2 Bass kernel for ragged-sequence attention.

reference computation (per batch b):
    energy[s] = sum_d key[s,b,:] . query[b,:]          (S=2048, D=512)
    w = softmax(energy) * mask;  w /= max(sum(w), eps)
    context = sum_s w[s] * value[s,b,:]
returns (context (B,D), w (B,S)).

Sharding: data parallel over batch, 4 batches per core on 8 cores, with a
rank-balanced batch->core assignment that equalizes ragged V traffic.

Raggedness: w is exactly zero for s >= len_b, so V rows beyond
ceil(len_b/128)*128 never contribute to the context.  Batches are sorted by
tile count and assigned round-robin (core c gets ranks {c, 8+c, 16+c, 24+c}),
so one compiled program with per-slot V tile counts Q = (TB[rank 0],
TB[rank 8], TB[rank 16], TB[rank 24]) serves every core.  The program is
compiled per Q-profile and cached.

Per-core dataflow:
  - q arrives pre-broadcast from the host ([128, 4*512], first on the ring).
  - K streams in as 1 MiB quad-batch tiles [128p, 4slot, 512d] (8 KiB runs).
  - energy via fused multiply+reduce (affine_mul_reduce) on DVE.
  - softmax: critical path to the context is only
    max (DVE reduce + gpsimd partition_all_reduce) -> exp (ACT, fused row
    sums) -> mask (DVE fused mask+mult+sum) -> PE transpose.  The
    normalizer r = 1/max(S_m, eps*Z) is computed off-path and folded into
    the final context copy (ACT scale) and the w output scale.
  - context: fp32 skinny matmuls over unnormalized weights, scheduled
    across the 4 PE column groups (tile_position) so up to 4 accumulation
    chains run concurrently; per-slot partial rows merged at the end.
"""
import contextlib
import os

import numpy as np

S, B, D = 2048, 32, 512
NCORES = 8
BL = B // NCORES          # 4 local batches (slots) per core
P = 128                   # partitions
NT = S // P               # 16 s-tiles of 128
EPS = 1e-12

_CACHE = {}
LAST_RESULT = None


def _chain_assignment(Q):
    """Assign each slot's context matmuls to PE column groups.  A slot
    larger than the balanced share is split across two groups; others get
    one group (no merge needed).  Returns group_of[(s, chain)] and the
    per-slot chain lists."""
    total = sum(Q)
    share = (total + BL - 1) // BL
    loads = [0] * BL
    chains = {}          # slot -> list of (group, t_list)
    for s in sorted(range(BL), key=lambda x: -Q[x]):
        ts = list(range(Q[s]))
        if Q[s] > share:
            h = (Q[s] + 1) // 2
            parts = [ts[:h], ts[h:]]
        else:
            parts = [ts]
        lst = []
        for part in parts:
            g = min(range(BL), key=lambda x: loads[x])
            loads[g] += len(part)
            lst.append((g, part))
        chains[s] = lst
    return chains


def _build(Q):
    from concourse import bacc, mybir
    import concourse.tile as tile
    from concourse import bass_isa

    F32 = mybir.dt.float32
    AX = mybir.AxisListType.X
    ALU = mybir.AluOpType
    EXP = mybir.ActivationFunctionType.Exp
    NV = sum(Q)

    nc = bacc.Bacc("TRN2", target_bir_lowering=False, debug=False,
                   num_devices=NCORES)

    K_in = nc.dram_tensor("K_in", [S, BL, D], F32, kind="ExternalInput").ap()
    V_in = nc.dram_tensor("V_in", [S, BL, D], F32, kind="ExternalInput").ap()
    qb_in = nc.dram_tensor("qb_in", [P, BL * D], F32, kind="ExternalInput").ap()
    th_in = nc.dram_tensor("th_in", [NT, BL], F32, kind="ExternalInput").ap()
    io_in = nc.dram_tensor("io_in", [NT, P], F32, kind="ExternalInput").ap()
    id_in = nc.dram_tensor("id_in", [P, P], F32, kind="ExternalInput").ap()
    ctx_out = nc.dram_tensor("ctx_out", [1, BL * D], F32,
                             kind="ExternalOutput").ap()
    w_out = nc.dram_tensor("w_out", [BL, NT, P], F32, kind="ExternalOutput").ap()

    # K quad tile t = [128p, 4slot, 512d] (1 MiB); V per-slot tile [128p, 512d]
    Kv = K_in.rearrange("(t p) b d -> t p b d", t=NT, p=P)
    Vv = V_in.rearrange("(t p) b d -> t b p d", t=NT, p=P)

    chains = _chain_assignment(Q)

    with tile.TileContext(nc) as tc:
        with contextlib.ExitStack() as ctx:
            kp = ctx.enter_context(tc.tile_pool(name="kp", bufs=8))
            vp = ctx.enter_context(tc.tile_pool(name="vp", bufs=NV))
            cb = ctx.enter_context(tc.tile_pool(name="cb", bufs=1))
            tp = ctx.enter_context(tc.tile_pool(name="tp", bufs=2))
            wp = ctx.enter_context(tc.tile_pool(name="wp", bufs=4))
            pp = ctx.enter_context(tc.tile_pool(name="pp", bufs=1, space="PSUM"))

            # --- q (pre-broadcast) + constants first on the sync ring ---
            qbsb = cb.tile([P, BL * D], F32)
            nc.sync.dma_start(qbsb[:], qb_in[:])
            ident = cb.tile([P, P], F32)
            nc.sync.dma_start(ident[:], id_in[:])
            iota = cb.tile([NT, P], F32)
            nc.sync.dma_start(iota[:], io_in[:])
            th = cb.tile([NT, BL], F32)
            nc.sync.dma_start(th[:], th_in[:])
            ones = cb.tile([NT, P], F32)
            nc.gpsimd.memset(ones[:], 1.0)

            # --- big-stream DMAs: K quads then ragged V, one HWDGE ring ---
            ktiles = []
            for t_i in range(NT):
                t = kp.tile([P, BL, D], F32, tag="kt")
                nc.sync.dma_start(t[:], Kv[t_i])
                ktiles.append(t)
            vtiles = {}
            for t_i in range(NT):
                for s in range(BL):
                    if t_i < Q[s]:
                        t = vp.tile([P, D], F32, tag="vt")
                        nc.sync.dma_start(t[:], Vv[t_i, s])
                        vtiles[(s, t_i)] = t

            # --- energy: E_b[:, t] = sum_d K[s,:]*q (fused DVE), K-paced ---
            Es = [cb.tile([P, NT + 1], F32, tag=f"E{b}", name=f"E{b}")
                  for b in range(BL)]
            for t in range(NT):
                for b in range(BL):
                    tmp = tp.tile([P, D], F32, tag="tmp")
                    nc.vector.affine_mul_reduce(
                        out=tmp[:], accum_out=Es[b][:, t:t + 1],
                        in0=ktiles[t][:, b, :],
                        in1=qbsb[:, b * D:(b + 1) * D],
                        scale=1.0, bias=0.0)

            # --- per-slot softmax ---
            wTs, r1s = [], []
            for b in range(BL):
                E = Es[b]
                # critical path: max -> exp -> mask -> transpose
                nc.vector.reduce_max(E[:, NT:NT + 1], E[:, 0:NT], axis=AX)
                ar = wp.tile([P, 1], F32, tag="ar")
                nc.gpsimd.partition_all_reduce(
                    ar[:], E[:, NT:NT + 1], channels=P,
                    reduce_op=bass_isa.ReduceOp.max)
                nm = wp.tile([NT, 1], F32, tag="nm")
                nc.vector.tensor_scalar_mul(nm[:], ar[0:NT, :], -1.0)

                Et_ps = pp.tile([NT, P], F32, tag="Et_ps", bufs=2)
                nc.tensor.transpose(Et_ps[:], E[:, 0:NT], ident[:])
                Et = wp.tile([NT, P], F32, tag="Et")
                nc.scalar.copy(Et[:], Et_ps[:])

                X = wp.tile([NT, P], F32, tag="X")
                scat = wp.tile([NT, 2], F32, tag="scat")
                nc.scalar.activation(X[:], Et[:], EXP, bias=nm[:], scale=1.0,
                                     accum_out=scat[:, 0:1])

                W1 = wp.tile([NT, P], F32, tag=f"W1{b}", name=f"W1{b}")
                nc.vector.scalar_tensor_tensor(
                    out=W1[:], in0=iota[:], scalar=th[:, b:b + 1], in1=X[:],
                    op0=ALU.is_lt, op1=ALU.mult, accum_out=scat[:, 1:2])

                wT_ps = pp.tile([P, NT], F32, tag="wT_ps")
                nc.tensor.transpose(wT_ps[:], W1[:], ident[0:NT, 0:NT])
                wT = wp.tile([P, NT], F32, tag=f"wT{b}", name=f"wT{b}")
                nc.scalar.copy(wT[:], wT_ps[:])
                wTs.append(wT)

                # off path: normalizer r = 1/max(S_m, eps*Z), w output
                ss_ps = pp.tile([1, 2], F32, tag="sm_ps", name="ss_ps")
                nc.tensor.matmul(ss_ps[:], ones[0:NT, 0:1], scat[:],
                                 start=True, stop=True)
                ss = wp.tile([1, 2], F32, tag="ss")
                nc.scalar.copy(ss[:], ss_ps[:])
                d1 = wp.tile([1, 1], F32, tag="d1")
                nc.vector.tensor_scalar_mul(d1[:], ss[:, 0:1], EPS)
                d2 = wp.tile([1, 1], F32, tag="d2")
                nc.vector.tensor_scalar_max(d2[:], ss[:, 1:2], d1[:])
                r1 = wp.tile([1, 1], F32, tag=f"r1{b}", name=f"r1{b}")
                nc.vector.reciprocal(r1[:], d2[:])
                r1s.append(r1)
                r_ps = pp.tile([NT, 1], F32, tag="sm_ps", name="r_ps")
                nc.tensor.matmul(r_ps[:], ones[0:1, 0:NT], r1[:],
                                 start=True, stop=True)
                r16 = wp.tile([NT, 1], F32, tag="r16")
                nc.scalar.copy(r16[:], r_ps[:])
                W = wp.tile([NT, P], F32, tag=f"W{b}", name=f"W{b}")
                nc.vector.tensor_scalar_mul(W[:], W1[:], r16[:])
                nc.gpsimd.dma_start(w_out[b], W[:])

            # --- context: chains across PE column groups, V-arrival order ---
            cps = [pp.tile([P, D], F32, tag="ctx_ps", bufs=BL, name=f"cps{b}")
                   for b in range(BL)]
            group_of = {}           # (s, t) -> group
            first_last = {}         # (s, g) -> (first_t, last_t)
            for s in range(BL):
                for g, ts in chains[s]:
                    for t in ts:
                        group_of[(s, t)] = g
                    first_last[(s, g)] = (ts[0], ts[-1])
            for t in range(NT):
                for s in range(BL):
                    if t < Q[s]:
                        g = group_of[(s, t)]
                        ft, lt = first_last[(s, g)]
                        nc.tensor.matmul(
                            cps[s][32 * g:32 * g + 1, :],
                            wTs[s][:, t:t + 1], vtiles[(s, t)][:],
                            start=(t == ft), stop=(t == lt),
                            tile_position=(0, 32 * g))

            # merge partial chains, scale by r, single output DMA
            cs_row = cb.tile([1, BL * D], F32)
            for s in range(BL):
                gl = chains[s]
                dst = cs_row[0:1, s * D:(s + 1) * D]
                if len(gl) == 1:
                    g = gl[0][0]
                    nc.scalar.mul(dst, cps[s][32 * g:32 * g + 1, :], r1s[s][:])
                else:
                    gA, gB = gl[0][0], gl[1][0]
                    mgB = wp.tile([1, D], F32, tag=f"mgB{s}", name=f"mgB{s}")
                    nc.scalar.copy(mgB[:], cps[s][32 * gB:32 * gB + 1, :])
                    mg = wp.tile([1, D], F32, tag=f"mg{s}", name=f"mg{s}")
                    nc.vector.tensor_add(mg[:], cps[s][32 * gA:32 * gA + 1, :],
                                         mgB[:])
                    nc.scalar.mul(dst, mg[:], r1s[s][:])
            nc.sync.dma_start(ctx_out[:], cs_row[:])

    nc.compile()
    return nc


def _get_nc(Q):
    if Q not in _CACHE:
        _CACHE[Q] = _build(Q)
    return _CACHE[Q]


def _maybe_register_trace_hook():
    """run_bass_kernel_spmd(trace=True) under axon needs antenv.axon_hooks;
    the RL image lacks it, so synthesize the module from trn_agent_boot."""
    import sys, types
    if "antenv.axon_hooks" in sys.modules:
        return
    try:
        import trn_agent_boot.trn_boot as tb
        hook = tb._ntff_profile_via_ctypes('/opt/axon/libaxon_pjrt.so')
        mod = types.ModuleType('antenv.axon_hooks')
        mod.get_axon_ntff_profile_hook = lambda: hook
        mod.set_axon_ntff_profile_hook = lambda h: None
        sys.modules['antenv.axon_hooks'] = mod
    except Exception:
        pass


def kernel(key, value, query, encoder_condensed_lens):
    global LAST_RESULT
    key = np.asarray(key, dtype=np.float32)
    value = np.asarray(value, dtype=np.float32)
    query = np.asarray(query, dtype=np.float32)
    lens = np.asarray(encoder_condensed_lens).astype(np.int64).reshape(B)

    # rank-balanced assignment: sort batches by V tile count (desc); core c
    # slot k <- rank 8k+c.  Q[k] = max tile count within slot k = rank 8k.
    TB = np.maximum(np.ceil(lens / P).astype(int), 1)
    order = np.argsort(-TB, kind="stable")
    Q = tuple(int(TB[order[8 * k]]) for k in range(BL))

    iota_np = np.tile(np.arange(P, dtype=np.float32)[None, :], (NT, 1))
    ident_np = np.eye(P, dtype=np.float32)

    in_maps = []
    batch_of = np.empty((NCORES, BL), dtype=int)
    for c in range(NCORES):
        bsel = [int(order[8 * k + c]) for k in range(BL)]
        batch_of[c] = bsel
        th = np.empty((NT, BL), dtype=np.float32)
        for k in range(BL):
            th[:, k] = lens[bsel[k]] - P * np.arange(NT)
        qb = np.tile(query[bsel, :].reshape(1, BL * D), (P, 1))
        in_maps.append({
            "K_in": np.ascontiguousarray(key[:, bsel, :]),
            "V_in": np.ascontiguousarray(value[:, bsel, :]),
            "qb_in": qb,
            "th_in": th,
            "io_in": iota_np,
            "id_in": ident_np,
        })

    nc = _get_nc(Q)
    from concourse import bass_utils
    trace = bool(os.environ.get("BASS_TRACE"))
    if trace:
        _maybe_register_trace_hook()
    res = bass_utils.run_bass_kernel_spmd(
        nc, in_maps, core_ids=list(range(NCORES)), trace=trace)
    LAST_RESULT = res

    context = np.empty((B, D), dtype=np.float32)
    w = np.empty((B, S), dtype=np.float32)
    for c in range(NCORES):
        r = res.results[c]
        ctxc = r["ctx_out"].reshape(BL, D)
        for k in range(BL):
            context[batch_of[c][k]] = ctxc[k]
            w[batch_of[c][k]] = r["w_out"][k].reshape(S)
    return context, w
